# revision 13
# baseline (speedup 1.0000x reference)
"""Trainium2 SPMD kernel: StyleGAN2-style modulated conv (Conv2dWeightModulate).

Reference math (per batch sample b):
    w0        = weight * RC                       (equalized-lr scale)
    ws        = w0 * style[b][None,:,None,None]   (per-input-channel modulation)
    sigma_inv = rsqrt(sum_{I,K,K} ws^2 + eps)     (per-output-channel demodulation)
    out[b]    = conv2d(x[b], ws * sigma_inv, pad=1)

Because the modulation is a per-input-channel scale and conv is linear, this
factorizes into ops with a SHARED weight across the batch:
    out[b] = sigma_inv[b,:] * conv2d(x[b] * (style[b]*RC), weight)
    sigma_inv[b,o] = rsqrt(RC^2 * sum_{i,t} weight[o,i,t]^2 * style[b,i]^2 + eps)

Sharding: data-parallel over batch: 8 samples -> 8 NeuronCores, weight
replicated (the groups=b conv factorizes exactly across the batch).

Schedule notes (from ntff trace analysis):
  - DMA rings kick ~8.2us (fixed); throughput is per-partition-segment-size
    bound: 2KB segments ~90GB/s aggregate, 4KB ~250, 6KB ~290, 12KB ~390.
    Rings balanced so every transfer completes ~2us+ before consumption:
      sync ring:   style, wt0 taps0-2, wt0 taps3-8, x1, x2, x3, outs
      scalar ring: x0 rows0-15, x0 rows16-31, wt1, wt2, wt3
    First conv matmul ~11.3us (vs 14.8 baseline).
  - PE p-states: tiny warmup matmuls bridge the dispatch ramp until conv
    deps land; HAM grants full clock ~5.2us after sustained matmul activity
    starts, then duty-cycles ~37.5us full / ~3.4us half. Big primer matmuls
    drain the duty budget early - use tiny ones only.
  - conv = 9 taps x 4 input-channel chunks of accumulated 128x128 @ 128x512
    fp32r matmuls (full-rate fp32 path), PSUM groups = 4 out-chunks x 2
    pixel halves. Wave A (h=0) uses banks 0-3; wave B (h=1) groups 4-6 use
    fresh banks 5-7 and only group 7 reuses bank 0 (freed by wave A's first
    flush ~26us earlier) -> zero PE stalls at the transition, and sigma
    finalization is pulled off the critical path entirely (flushes happen
    lazily once sig_t lands mid-wave-B).
  - sigma: ACT squares each weight chunk (bf16), DVE sums taps, PE reduces
    against style^2 via [128,1]-lhsT matmuls emitted at wave-B chunk
    boundaries into a [1,512] PSUM row; ACT sqrt (table prefetched by a
    dummy op after the last square) -> PE-transpose -> DVE reciprocal.
  - PSUM->SBUF flushes (scale by sigma_inv) on DVE into per-h [128,2048]
    staging tiles; out DMAs per oc-pair ([128,1024], 4KB segments).
"""

from contextlib import ExitStack

import numpy as np

import concourse.bass as bass
import concourse.tile as tile
from concourse import bacc, mybir
from concourse.bass_utils import run_bass_kernel_spmd

B = 8
CIN = 512
COUT = 512
KK = 3
H = 32
W = 32
PIX = H * W
NCH = 4  # channel chunks of 128
TAPS = KK * KK
RC = float(1.0 / np.sqrt(CIN * KK * KK))
EPS = 1e-8
F32 = mybir.dt.float32
F32R = mybir.dt.float32r
BF16 = mybir.dt.bfloat16
AF = mybir.ActivationFunctionType

# test.py toggles these; the grading harness just calls kernel().
TRACE = False
LAST_RESULTS = None


def _body(ctx, tc, x_d, st_d, wt_d, out_d):
    nc = tc.nc
    const = ctx.enter_context(tc.tile_pool(name="const", bufs=1))
    wpool = ctx.enter_context(tc.tile_pool(name="wpool", bufs=1))
    xpool = ctx.enter_context(tc.tile_pool(name="xpool", bufs=1))
    sqpool = ctx.enter_context(tc.tile_pool(name="sqpool", bufs=3))
    opool = ctx.enter_context(tc.tile_pool(name="opool", bufs=1))
    psum = ctx.enter_context(
        tc.tile_pool(name="psum", bufs=1, space=bass.MemorySpace.PSUM)
    )
    sigpsum = ctx.enter_context(
        tc.tile_pool(name="sigpsum", bufs=1, space=bass.MemorySpace.PSUM)
    )

    # --- tiles ---
    st = const.tile([128, NCH], F32, tag="st")
    st_rc = const.tile([128, NCH], F32, tag="st_rc")
    st2 = const.tile([128, NCH], BF16, tag="st2")

    wt = [
        wpool.tile([128, TAPS, COUT], F32R, tag=f"wt{c}", name=f"wt{c}")
        for c in range(NCH)
    ]
    xs = []
    for c in range(NCH):
        xc = xpool.tile([128, H + 2, W + 2], F32R, tag=f"xs{c}", name=f"xs{c}")
        xs.append(xc)
    xst0 = xpool.tile([128, H, W], F32, tag="xst0", name="xst0")
    xst123 = xpool.tile([128, NCH - 1, PIX], F32, tag="xst123", name="xst123")

    warm_src = const.tile([1, 128], F32R, tag="warm_src")
    ones_r = const.tile([1, 1], F32R, tag="ones_r")
    warm_l = const.tile([128, 128], F32R, tag="warm_l")
    warm_r = const.tile([128, 512], F32R, tag="warm_r")

    # one PSUM bank shared by: warmup dst, sigma accumulation row, transposes
    sigbank = sigpsum.tile([128, 512], F32, tag="sigbank")
    sig_ps = sigbank[0:1, :]
    warm_ps = sigbank[0:1, 0:128]

    eps_b = const.tile([1, 1], F32, tag="eps_b")
    ones_t = const.tile([1, 1], F32, tag="ones_t")
    sqrt_dummy = const.tile([1, 1], F32, tag="sqrt_dummy")
    sig_sq = const.tile([1, COUT], F32, tag="sig_sq")
    sig_sd = const.tile([128, NCH], F32, tag="sig_sd")
    sig_t = const.tile([128, NCH], F32, tag="sig_t")
    # per-h output staging: oc writes cols [oc*512:(oc+1)*512]
    ob_h = [
        opool.tile([128, NCH * 512], F32, tag=f"ob{h}", name=f"ob{h}")
        for h in range(2)
    ]

    # --- early memsets (no data deps) ---
    nc.gpsimd.memset(warm_src[:].bitcast(F32), 1.0)
    nc.gpsimd.memset(ones_r[:].bitcast(F32), 1.0)
    nc.gpsimd.memset(warm_l[:].bitcast(F32), 0.0)
    nc.gpsimd.memset(warm_r[:].bitcast(F32), 0.0)
    nc.vector.memset(eps_b[:], EPS)
    nc.vector.memset(ones_t[:], 1.0)
    nc.vector.memset(sqrt_dummy[:], 1.0)
    for c in range(NCH):
        eng = nc.vector if c == 0 else nc.gpsimd
        v = xs[c][:].bitcast(F32)
        eng.memset(v[:, 0, :], 0.0)
        eng.memset(v[:, H + 1, :], 0.0)
        eng.memset(v[:, 1 : H + 1, 0], 0.0)
        eng.memset(v[:, 1 : H + 1, W + 1], 0.0)

    # --- DMA issue order. The 16 DMA engines are one shared pool: keep
    # per-partition segments >=4KB anywhere near the critical window. ---
    # sync ring
    nc.sync.dma_start(st[:], st_d[:])
    nc.sync.dma_start(wt[0][:, 0:3], wt_d[:, 0, 0:3])
    nc.sync.dma_start(wt[0][:, 3:9], wt_d[:, 0, 3:9])
    nc.sync.dma_start(xst123[:], x_d[1:NCH].rearrange("c p q -> p c q"))
    # scalar ring (parallel; its ACT table load does not block DMA issue)
    nc.scalar.dma_start(xst0[:], x_d[0].rearrange("p (h w) -> p h w", h=H))
    for c in range(1, NCH):
        nc.scalar.dma_start(wt[c][:], wt_d[:, c])

    # --- style scales (DVE; after style DMA) ---
    nc.vector.tensor_scalar_mul(st_rc[:], st[:], RC)
    nc.vector.tensor_mul(st2[:], st[:], st[:])

    # --- x modulation (DVE); x0 scaled in halves so conv's first taps can
    # start ~0.4us after x0 lands ---
    nc.vector.tensor_scalar_mul(
        xs[0][:, 1:17, 1 : W + 1], xst0[:, 0:16, :], st_rc[:, 0:1]
    )
    nc.vector.tensor_scalar_mul(
        xs[0][:, 17 : H + 1, 1 : W + 1], xst0[:, 16:32, :], st_rc[:, 0:1]
    )
    for c in range(1, NCH):
        nc.vector.tensor_scalar_mul(
            xs[c][:, 1 : H + 1, 1 : W + 1],
            xst123[:, c - 1].rearrange("p (h w) -> p h w", h=H),
            st_rc[:, c : c + 1],
        )

    # --- PE warmup: tiny matmuls bridge the p-state ramp, then a few
    # full-size primers connect seamlessly into the conv stream so the HAM
    # high-activity detector starts its ~6us count before the first conv mm
    for _ in range(28):
        nc.tensor.matmul(warm_ps, ones_r[:], warm_src[:], start=True, stop=True)
    for _ in range(6):
        nc.tensor.matmul(sigbank[:], warm_l[:], warm_r[:], start=True, stop=True)

    # wave A groups: (oc, h=0) -> banks pc0-3; wave B: g4-6 fresh banks, g7
    # reuses pc0 (freed by wave A's flush long before g7 starts)
    wave_a = [(oc, 0) for oc in range(NCH)]
    wave_b = [(oc, 1) for oc in range(NCH)]
    pc = {
        g: psum.tile([128, 512], F32, tag=f"pc{i}", name=f"pc{i}")
        for i, g in enumerate(wave_a)
    }

    def conv_mm(g, c, t, start, stop):
        oc, h = g
        dy, dx = t // 3, t % 3
        h0 = h * 16
        nc.tensor.matmul(
            pc[g][:],
            wt[c][:, t, oc * 128 : (oc + 1) * 128],
            xs[c][:, dy + h0 : dy + h0 + 16, dx : dx + W],
            start=start,
            stop=stop,
        )

    # Per-chunk sum over taps of squared weights (ACT squares, DVE adds):
    # cuts the PE cost of the sigma reduction from 36 matmuls to 4.
    w2s = {}

    def sig_squares(c):
        parts = []
        for t in range(TAPS):
            w2 = sqpool.tile([128, COUT], BF16, tag=f"w2_{t % 3}", name="w2")
            nc.scalar.activation(w2[:], wt[c][:, t], AF.Square)
            parts.append(w2)
            if t == 1:
                acc = sqpool.tile([128, COUT], BF16, tag=f"w2s{c}", name="w2s")
                nc.vector.tensor_add(acc[:], parts[0][:], parts[1][:])
            elif t > 1:
                nc.vector.tensor_add(acc[:], acc[:], parts[-1][:])
        w2s[c] = acc
        if c == NCH - 1:
            # prefetch the Sqrt ACT table (1.3us load) off the critical path
            nc.scalar.activation(sqrt_dummy[:], sqrt_dummy[:], AF.Sqrt)

    def sig_mm(c):
        nc.tensor.matmul(
            sig_ps, st2[:, c : c + 1], w2s[c][:], start=(c == 0), stop=(c == NCH - 1)
        )

    def sig_finalize():
        # sqrt(RC^2*q + eps) [1,512] -> PE-transpose -> [128,4] -> reciprocal
        nc.scalar.activation(
            sig_sq[:], sig_ps, AF.Sqrt, bias=eps_b[:], scale=RC * RC
        )
        for oc in range(NCH):
            nc.tensor.transpose(
                sigbank[:, oc : oc + 1],
                sig_sq[0:1, oc * 128 : (oc + 1) * 128],
                ones_t[:],
            )
        nc.vector.tensor_copy(sig_sd[:], sigbank[:, 0:NCH])
        nc.vector.reciprocal(sig_t[:], sig_sd[:])

    def flush(g):
        # PSUM -> SBUF scaled by sigma_inv, on DVE; waits on sig_t which lands
        # mid-wave-B - only g7's bank reuse (~26us later) depends on it.
        oc, h = g
        nc.vector.tensor_scalar_mul(
            ob_h[h][:, oc * 512 : (oc + 1) * 512], pc[g][:], sig_t[:, oc : oc + 1]
        )

    def out_dma(h, pair):
        nc.sync.dma_start(
            out_d[h, pair, :, :],
            ob_h[h][:, pair * 1024 : (pair + 1) * 1024].rearrange(
                "p (a q) -> p a q", a=2
            ),
        )

    # --- wave A (h=0): tap-major, group-minor; squares pipeline alongside ---
    for c in range(NCH):
        sig_squares(c)
        for t in range(TAPS):
            for g in wave_a:
                conv_mm(g, c, t, c == 0 and t == 0, c == NCH - 1 and t == TAPS - 1)

    # --- wave B (h=1): per-group serial chains; sigma reduction at g4 chunk
    # boundaries, finalize after g4. All flush emissions come after
    # sig_finalize so sig_t's write precedes every flush in DVE program
    # order (the tile tracker cannot order a read before a future write). ---
    for gi, g in enumerate(wave_b):
        tag = "pc0" if gi == 3 else f"pcB{gi}"
        pc[g] = psum.tile([128, 512], F32, tag=tag, name=f"pcb{gi}")
        k = 0
        for c in range(NCH):
            if gi == 0:
                sig_mm(c)
            for t in range(TAPS):
                conv_mm(g, c, t, k == 0, k == TAPS * NCH - 1)
                k += 1
        if gi == 0:
            sig_finalize()
            for ga in wave_a:
                flush(ga)
            out_dma(0, 0)
            out_dma(0, 1)
        flush(g)
        if gi == 1:
            out_dma(1, 0)
        elif gi == 3:
            out_dma(1, 1)


_CACHE = None


def _get_compiled():
    global _CACHE
    if _CACHE is None:
        nc = bacc.Bacc(
            "TRN2", target_bir_lowering=False, debug=False, num_devices=B
        )
        x_d = nc.dram_tensor("x", [NCH, 128, PIX], F32, kind="ExternalInput").ap()
        st_d = nc.dram_tensor("style", [128, NCH], F32, kind="ExternalInput").ap()
        wt_d = nc.dram_tensor(
            "wt", [128, NCH, TAPS, COUT], F32R, kind="ExternalInput"
        ).ap()
        # out layout: [h, oc_pair, 128, 2, 512] -> host reassembles
        out_d = nc.dram_tensor(
            "out", [2, 2, 128, 2, 512], F32, kind="ExternalOutput"
        ).ap()
        with tile.TileContext(nc) as tc, ExitStack() as ctx:
            _body(ctx, tc, x_d, st_d, wt_d, out_d)
        nc.compile()
        _CACHE = nc
    return _CACHE


def kernel(x, style, weight):
    """x: (8,512,32,32) f32, style: (8,512) f32, weight: (512,512,3,3) f32
    -> (8,512,32,32) f32"""
    global LAST_RESULTS
    x = np.ascontiguousarray(np.asarray(x, dtype=np.float32))
    style = np.asarray(style, dtype=np.float32)
    weight = np.asarray(weight, dtype=np.float32)

    # Host-side layout only (no arithmetic): lhsT weight layout
    # wt[i_lo, c, t, o] = weight[o, c*128 + i_lo, t//3, t%3]
    wt = np.ascontiguousarray(
        weight.reshape(COUT, NCH, 128, TAPS).transpose(2, 1, 3, 0)
    )
    in_maps = []
    for b in range(B):
        in_maps.append(
            {
                "x": x[b].reshape(NCH, 128, PIX),
                "style": np.ascontiguousarray(style[b].reshape(NCH, 128).T),
                "wt": wt,
            }
        )

    nc = _get_compiled()
    res = run_bass_kernel_spmd(nc, in_maps, list(range(B)), trace=TRACE)
    LAST_RESULTS = res
    out = np.empty((B, COUT, H, W), dtype=np.float32)
    for b in range(B):
        # out HBM [h, pair, 128(cout_lo), j(oc in pair), 512(pix half)]
        o = res.results[b]["out"]
        o = o.transpose(1, 3, 2, 0, 4)  # -> [pair, j, cout_lo, h, q]
        out[b] = o.reshape(COUT, H, W)
    return out


# revision 14
# speedup vs baseline: 1.0693x; 1.0693x over previous
"""Trainium2 SPMD kernel: StyleGAN2-style modulated conv (Conv2dWeightModulate).

Reference math (per batch sample b):
    w0        = weight * RC                       (equalized-lr scale)
    ws        = w0 * style[b][None,:,None,None]   (per-input-channel modulation)
    sigma_inv = rsqrt(sum_{I,K,K} ws^2 + eps)     (per-output-channel demodulation)
    out[b]    = conv2d(x[b], ws * sigma_inv, pad=1)

Because the modulation is a per-input-channel scale and conv is linear, this
factorizes into ops with a SHARED weight across the batch:
    out[b] = sigma_inv[b,:] * conv2d(x[b] * (style[b]*RC), weight)
    sigma_inv[b,o] = rsqrt(RC^2 * sum_{i,t} weight[o,i,t]^2 * style[b,i]^2 + eps)

Sharding: data-parallel over batch: 8 samples -> 8 NeuronCores, weight
replicated (the groups=b conv factorizes exactly across the batch).

Schedule notes (from ntff trace analysis):
  - DMA rings kick ~8.2us (fixed); throughput is per-partition-segment-size
    bound: 2KB segments ~90GB/s aggregate, 4KB ~250, 6KB ~290, 12KB ~390.
    Rings balanced so every transfer completes ~2us+ before consumption:
      sync ring:   style, wt0 taps0-2, wt0 taps3-8, x1, x2, x3, outs
      scalar ring: x0 rows0-15, x0 rows16-31, wt1, wt2, wt3
    First conv matmul ~11.3us (vs 14.8 baseline).
  - PE p-states: tiny warmup matmuls bridge the dispatch ramp until conv
    deps land; HAM grants full clock ~5.2us after sustained matmul activity
    starts, then duty-cycles ~37.5us full / ~3.4us half. Big primer matmuls
    drain the duty budget early - use tiny ones only.
  - conv = 9 taps x 4 input-channel chunks of accumulated 128x128 @ 128x512
    fp32r matmuls (full-rate fp32 path), PSUM groups = 4 out-chunks x 2
    pixel halves. Wave A (h=0) uses banks 0-3; wave B (h=1) groups 4-6 use
    fresh banks 5-7 and only group 7 reuses bank 0 (freed by wave A's first
    flush ~26us earlier) -> zero PE stalls at the transition, and sigma
    finalization is pulled off the critical path entirely (flushes happen
    lazily once sig_t lands mid-wave-B).
  - sigma: ACT squares each weight chunk (bf16), DVE sums taps, PE reduces
    against style^2 via [128,1]-lhsT matmuls emitted at wave-B chunk
    boundaries into a [1,512] PSUM row; ACT sqrt (table prefetched by a
    dummy op after the last square) -> PE-transpose -> DVE reciprocal.
  - PSUM->SBUF flushes (scale by sigma_inv) on DVE into per-h [128,2048]
    staging tiles; out DMAs per oc-pair ([128,1024], 4KB segments).
"""

from contextlib import ExitStack

import numpy as np

import concourse.bass as bass
import concourse.tile as tile
from concourse import bacc, mybir
from concourse.bass_utils import run_bass_kernel_spmd

B = 8
CIN = 512
COUT = 512
KK = 3
H = 32
W = 32
PIX = H * W
NCH = 4  # channel chunks of 128
TAPS = KK * KK
RC = float(1.0 / np.sqrt(CIN * KK * KK))
EPS = 1e-8
F32 = mybir.dt.float32
F32R = mybir.dt.float32r
BF16 = mybir.dt.bfloat16
AF = mybir.ActivationFunctionType

# test.py toggles these; the grading harness just calls kernel().
TRACE = False
LAST_RESULTS = None


def _body(ctx, tc, x_d, st_d, wt_d, out_d):
    nc = tc.nc
    const = ctx.enter_context(tc.tile_pool(name="const", bufs=1))
    wpool = ctx.enter_context(tc.tile_pool(name="wpool", bufs=1))
    xpool = ctx.enter_context(tc.tile_pool(name="xpool", bufs=1))
    sqpool = ctx.enter_context(tc.tile_pool(name="sqpool", bufs=3))
    opool = ctx.enter_context(tc.tile_pool(name="opool", bufs=1))
    psum = ctx.enter_context(
        tc.tile_pool(name="psum", bufs=1, space=bass.MemorySpace.PSUM)
    )
    sigpsum = ctx.enter_context(
        tc.tile_pool(name="sigpsum", bufs=1, space=bass.MemorySpace.PSUM)
    )

    # --- tiles ---
    st = const.tile([128, NCH], F32, tag="st")
    st_rc = const.tile([128, NCH], F32, tag="st_rc")
    st2 = const.tile([128, NCH], BF16, tag="st2")

    wt = [
        wpool.tile([128, TAPS, COUT], F32R, tag=f"wt{c}", name=f"wt{c}")
        for c in range(NCH)
    ]
    xs = []
    for c in range(NCH):
        xc = xpool.tile([128, H + 2, W + 2], F32R, tag=f"xs{c}", name=f"xs{c}")
        xs.append(xc)
    xst0 = xpool.tile([128, H, W], F32, tag="xst0", name="xst0")
    xst123 = xpool.tile([128, NCH - 1, PIX], F32, tag="xst123", name="xst123")

    warm_src = const.tile([1, 128], F32R, tag="warm_src")
    ones_r = const.tile([1, 1], F32R, tag="ones_r")
    warm_l = const.tile([128, 128], F32R, tag="warm_l")
    warm_r = const.tile([128, 512], F32R, tag="warm_r")

    # one PSUM bank shared by: warmup dst, sigma accumulation row, transposes
    sigbank = sigpsum.tile([128, 512], F32, tag="sigbank")
    sig_ps = sigbank[0:1, :]
    warm_ps = sigbank[0:1, 0:128]

    eps_b = const.tile([1, 1], F32, tag="eps_b")
    ones_t = const.tile([1, 1], F32, tag="ones_t")
    sqrt_dummy = const.tile([1, 1], F32, tag="sqrt_dummy")
    sig_sq = const.tile([1, COUT], F32, tag="sig_sq")
    sig_sd = const.tile([128, NCH], F32, tag="sig_sd")
    sig_t = const.tile([128, NCH], F32, tag="sig_t")
    # per-h output staging: oc writes cols [oc*512:(oc+1)*512]
    ob_h = [
        opool.tile([128, NCH * 512], F32, tag=f"ob{h}", name=f"ob{h}")
        for h in range(2)
    ]

    # --- early memsets (no data deps) ---
    nc.gpsimd.memset(warm_src[:].bitcast(F32), 1.0)
    nc.gpsimd.memset(ones_r[:].bitcast(F32), 1.0)
    nc.gpsimd.memset(warm_l[:].bitcast(F32), 0.0)
    nc.gpsimd.memset(warm_r[:].bitcast(F32), 0.0)
    nc.vector.memset(eps_b[:], EPS)
    nc.vector.memset(ones_t[:], 1.0)
    nc.vector.memset(sqrt_dummy[:], 1.0)
    for c in range(NCH):
        eng = nc.vector if c == 0 else nc.gpsimd
        v = xs[c][:].bitcast(F32)
        eng.memset(v[:, 0, :], 0.0)
        eng.memset(v[:, H + 1, :], 0.0)
        eng.memset(v[:, 1 : H + 1, 0], 0.0)
        eng.memset(v[:, 1 : H + 1, W + 1], 0.0)

    # --- DMA issue order. The 16 DMA engines are one shared pool: keep
    # per-partition segments >=4KB anywhere near the critical window. ---
    # sync ring
    nc.sync.dma_start(st[:], st_d[:])
    nc.sync.dma_start(wt[0][:, 0:3], wt_d[:, 0, 0:3])
    nc.sync.dma_start(wt[0][:, 3:9], wt_d[:, 0, 3:9])
    # scalar ring (parallel; its ACT table load does not block DMA issue).
    # x123 sits between x0 and wt1 so wt1 does not compete with wt0b's
    # window on the shared DMA-engine pool.
    nc.scalar.dma_start(xst0[:], x_d[0].rearrange("p (h w) -> p h w", h=H))
    nc.scalar.dma_start(xst123[:], x_d[1:NCH].rearrange("c p q -> p c q"))
    for c in range(1, NCH):
        nc.scalar.dma_start(wt[c][:], wt_d[:, c])

    # --- style scales (DVE; after style DMA) ---
    nc.vector.tensor_scalar_mul(st_rc[:], st[:], RC)
    nc.vector.tensor_mul(st2[:], st[:], st[:])

    # --- x modulation (DVE); x0 scaled in halves so conv's first taps can
    # start ~0.4us after x0 lands ---
    nc.vector.tensor_scalar_mul(
        xs[0][:, 1:17, 1 : W + 1], xst0[:, 0:16, :], st_rc[:, 0:1]
    )
    nc.vector.tensor_scalar_mul(
        xs[0][:, 17 : H + 1, 1 : W + 1], xst0[:, 16:32, :], st_rc[:, 0:1]
    )
    for c in range(1, NCH):
        nc.vector.tensor_scalar_mul(
            xs[c][:, 1 : H + 1, 1 : W + 1],
            xst123[:, c - 1].rearrange("p (h w) -> p h w", h=H),
            st_rc[:, c : c + 1],
        )

    # --- PE warmup: tiny matmuls bridge the p-state ramp until conv deps
    # land. Tiny ONLY: full-size primers pull the HAM grant earlier but
    # trigger duty-cycle oscillation that costs far more mid-stream. ---
    for _ in range(36):
        nc.tensor.matmul(warm_ps, ones_r[:], warm_src[:], start=True, stop=True)

    # wave A groups: (oc, h=0) -> banks pc0-3; wave B: g4-6 fresh banks, g7
    # reuses pc0 (freed by wave A's flush long before g7 starts)
    wave_a = [(oc, 0) for oc in range(NCH)]
    wave_b = [(oc, 1) for oc in range(NCH)]
    pc = {
        g: psum.tile([128, 512], F32, tag=f"pc{i}", name=f"pc{i}")
        for i, g in enumerate(wave_a)
    }

    def conv_mm(g, c, t, start, stop):
        oc, h = g
        dy, dx = t // 3, t % 3
        h0 = h * 16
        nc.tensor.matmul(
            pc[g][:],
            wt[c][:, t, oc * 128 : (oc + 1) * 128],
            xs[c][:, dy + h0 : dy + h0 + 16, dx : dx + W],
            start=start,
            stop=stop,
        )

    # Per-chunk sum over taps of squared weights (ACT squares, DVE adds):
    # cuts the PE cost of the sigma reduction from 36 matmuls to 4.
    w2s = {}

    def sig_squares(c):
        parts = []
        for t in range(TAPS):
            w2 = sqpool.tile([128, COUT], BF16, tag=f"w2_{t % 3}", name="w2")
            nc.scalar.activation(w2[:], wt[c][:, t], AF.Square)
            parts.append(w2)
            if t == 1:
                acc = sqpool.tile([128, COUT], BF16, tag=f"w2s{c}", name="w2s")
                nc.vector.tensor_add(acc[:], parts[0][:], parts[1][:])
            elif t > 1:
                nc.vector.tensor_add(acc[:], acc[:], parts[-1][:])
        w2s[c] = acc
        if c == NCH - 1:
            # prefetch the Sqrt ACT table (1.3us load) off the critical path
            nc.scalar.activation(sqrt_dummy[:], sqrt_dummy[:], AF.Sqrt)

    def sig_mm(c):
        nc.tensor.matmul(
            sig_ps, st2[:, c : c + 1], w2s[c][:], start=(c == 0), stop=(c == NCH - 1)
        )

    def sig_finalize():
        # sqrt(RC^2*q + eps) [1,512] -> PE-transpose -> [128,4] -> reciprocal
        nc.scalar.activation(
            sig_sq[:], sig_ps, AF.Sqrt, bias=eps_b[:], scale=RC * RC
        )
        for oc in range(NCH):
            nc.tensor.transpose(
                sigbank[:, oc : oc + 1],
                sig_sq[0:1, oc * 128 : (oc + 1) * 128],
                ones_t[:],
            )
        nc.vector.tensor_copy(sig_sd[:], sigbank[:, 0:NCH])
        nc.vector.reciprocal(sig_t[:], sig_sd[:])

    def flush(g):
        # PSUM -> SBUF scaled by sigma_inv, on DVE; waits on sig_t which lands
        # mid-wave-B - only g7's bank reuse (~26us later) depends on it.
        oc, h = g
        nc.vector.tensor_scalar_mul(
            ob_h[h][:, oc * 512 : (oc + 1) * 512], pc[g][:], sig_t[:, oc : oc + 1]
        )

    def out_dma(h, pair):
        nc.sync.dma_start(
            out_d[h, pair, :, :],
            ob_h[h][:, pair * 1024 : (pair + 1) * 1024].rearrange(
                "p (a q) -> p a q", a=2
            ),
        )

    # --- wave A (h=0): tap-major, group-minor; squares pipeline alongside ---
    for c in range(NCH):
        sig_squares(c)
        for t in range(TAPS):
            for g in wave_a:
                conv_mm(g, c, t, c == 0 and t == 0, c == NCH - 1 and t == TAPS - 1)

    # --- wave B (h=1): per-group serial chains; sigma reduction at g4 chunk
    # boundaries, finalize after g4. All flush emissions come after
    # sig_finalize so sig_t's write precedes every flush in DVE program
    # order (the tile tracker cannot order a read before a future write). ---
    for gi, g in enumerate(wave_b):
        tag = "pc0" if gi == 3 else f"pcB{gi}"
        pc[g] = psum.tile([128, 512], F32, tag=tag, name=f"pcb{gi}")
        k = 0
        for c in range(NCH):
            if gi == 0:
                sig_mm(c)
            for t in range(TAPS):
                conv_mm(g, c, t, k == 0, k == TAPS * NCH - 1)
                k += 1
        if gi == 0:
            sig_finalize()
            for ga in wave_a:
                flush(ga)
            out_dma(0, 0)
            out_dma(0, 1)
        flush(g)
        if gi == 1:
            out_dma(1, 0)
        elif gi == 3:
            out_dma(1, 1)


_CACHE = None


def _get_compiled():
    global _CACHE
    if _CACHE is None:
        nc = bacc.Bacc(
            "TRN2", target_bir_lowering=False, debug=False, num_devices=B
        )
        x_d = nc.dram_tensor("x", [NCH, 128, PIX], F32, kind="ExternalInput").ap()
        st_d = nc.dram_tensor("style", [128, NCH], F32, kind="ExternalInput").ap()
        wt_d = nc.dram_tensor(
            "wt", [128, NCH, TAPS, COUT], F32R, kind="ExternalInput"
        ).ap()
        # out layout: [h, oc_pair, 128, 2, 512] -> host reassembles
        out_d = nc.dram_tensor(
            "out", [2, 2, 128, 2, 512], F32, kind="ExternalOutput"
        ).ap()
        with tile.TileContext(nc) as tc, ExitStack() as ctx:
            _body(ctx, tc, x_d, st_d, wt_d, out_d)
        nc.compile()
        _CACHE = nc
    return _CACHE


def kernel(x, style, weight):
    """x: (8,512,32,32) f32, style: (8,512) f32, weight: (512,512,3,3) f32
    -> (8,512,32,32) f32"""
    global LAST_RESULTS
    x = np.ascontiguousarray(np.asarray(x, dtype=np.float32))
    style = np.asarray(style, dtype=np.float32)
    weight = np.asarray(weight, dtype=np.float32)

    # Host-side layout only (no arithmetic): lhsT weight layout
    # wt[i_lo, c, t, o] = weight[o, c*128 + i_lo, t//3, t%3]
    wt = np.ascontiguousarray(
        weight.reshape(COUT, NCH, 128, TAPS).transpose(2, 1, 3, 0)
    )
    in_maps = []
    for b in range(B):
        in_maps.append(
            {
                "x": x[b].reshape(NCH, 128, PIX),
                "style": np.ascontiguousarray(style[b].reshape(NCH, 128).T),
                "wt": wt,
            }
        )

    nc = _get_compiled()
    res = run_bass_kernel_spmd(nc, in_maps, list(range(B)), trace=TRACE)
    LAST_RESULTS = res
    out = np.empty((B, COUT, H, W), dtype=np.float32)
    for b in range(B):
        # out HBM [h, pair, 128(cout_lo), j(oc in pair), 512(pix half)]
        o = res.results[b]["out"]
        o = o.transpose(1, 3, 2, 0, 4)  # -> [pair, j, cout_lo, h, q]
        out[b] = o.reshape(COUT, H, W)
    return out


# revision 15
# speedup vs baseline: 1.0998x; 1.0285x over previous
"""Trainium2 SPMD kernel: StyleGAN2-style modulated conv (Conv2dWeightModulate).

Reference math (per batch sample b):
    w0        = weight * RC                       (equalized-lr scale)
    ws        = w0 * style[b][None,:,None,None]   (per-input-channel modulation)
    sigma_inv = rsqrt(sum_{I,K,K} ws^2 + eps)     (per-output-channel demodulation)
    out[b]    = conv2d(x[b], ws * sigma_inv, pad=1)

Because the modulation is a per-input-channel scale and conv is linear, this
factorizes into ops with a SHARED weight across the batch:
    out[b] = sigma_inv[b,:] * conv2d(x[b] * (style[b]*RC), weight)
    sigma_inv[b,o] = rsqrt(RC^2 * sum_{i,t} weight[o,i,t]^2 * style[b,i]^2 + eps)

Sharding: data-parallel over batch: 8 samples -> 8 NeuronCores, weight
replicated (the groups=b conv factorizes exactly across the batch).

Schedule notes (from ntff trace analysis):
  - DMA rings kick ~8.2us (fixed); throughput is per-partition-segment-size
    bound: 2KB segments ~90GB/s aggregate, 4KB ~250, 6KB ~290, 12KB ~390.
    Rings balanced so every transfer completes ~2us+ before consumption:
      sync ring:   style, wt0 taps0-2, wt0 taps3-8, x1, x2, x3, outs
      scalar ring: x0 rows0-15, x0 rows16-31, wt1, wt2, wt3
    First conv matmul ~11.3us (vs 14.8 baseline).
  - PE p-states: tiny warmup matmuls bridge the dispatch ramp until conv
    deps land; HAM grants full clock ~5.2us after sustained matmul activity
    starts, then duty-cycles ~37.5us full / ~3.4us half. Big primer matmuls
    drain the duty budget early - use tiny ones only.
  - conv = 9 taps x 4 input-channel chunks of accumulated 128x128 @ 128x512
    fp32r matmuls (full-rate fp32 path), PSUM groups = 4 out-chunks x 2
    pixel halves. Wave A (h=0) uses banks 0-3; wave B (h=1) groups 4-6 use
    fresh banks 5-7 and only group 7 reuses bank 0 (freed by wave A's first
    flush ~26us earlier) -> zero PE stalls at the transition, and sigma
    finalization is pulled off the critical path entirely (flushes happen
    lazily once sig_t lands mid-wave-B).
  - sigma: ACT squares each weight chunk (bf16), DVE sums taps, PE reduces
    against style^2 via [128,1]-lhsT matmuls emitted at wave-B chunk
    boundaries into a [1,512] PSUM row; ACT sqrt (table prefetched by a
    dummy op after the last square) -> PE-transpose -> DVE reciprocal.
  - PSUM->SBUF flushes (scale by sigma_inv) on DVE into per-h [128,2048]
    staging tiles; out DMAs per oc-pair ([128,1024], 4KB segments).
"""

from contextlib import ExitStack

import numpy as np

import concourse.bass as bass
import concourse.tile as tile
from concourse import bacc, mybir
from concourse.bass_utils import run_bass_kernel_spmd

B = 8
CIN = 512
COUT = 512
KK = 3
H = 32
W = 32
PIX = H * W
NCH = 4  # channel chunks of 128
TAPS = KK * KK
RC = float(1.0 / np.sqrt(CIN * KK * KK))
EPS = 1e-8
F32 = mybir.dt.float32
F32R = mybir.dt.float32r
BF16 = mybir.dt.bfloat16
AF = mybir.ActivationFunctionType

# test.py toggles these; the grading harness just calls kernel().
TRACE = False
LAST_RESULTS = None


def _body(ctx, tc, x_d, st_d, wt_d, out_d):
    nc = tc.nc
    const = ctx.enter_context(tc.tile_pool(name="const", bufs=1))
    wpool = ctx.enter_context(tc.tile_pool(name="wpool", bufs=1))
    xpool = ctx.enter_context(tc.tile_pool(name="xpool", bufs=1))
    sqpool = ctx.enter_context(tc.tile_pool(name="sqpool", bufs=3))
    opool = ctx.enter_context(tc.tile_pool(name="opool", bufs=1))
    psum = ctx.enter_context(
        tc.tile_pool(name="psum", bufs=1, space=bass.MemorySpace.PSUM)
    )
    sigpsum = ctx.enter_context(
        tc.tile_pool(name="sigpsum", bufs=1, space=bass.MemorySpace.PSUM)
    )

    # --- tiles ---
    st = const.tile([128, NCH], F32, tag="st")
    st_rc = const.tile([128, NCH], F32, tag="st_rc")
    st2 = const.tile([128, NCH], BF16, tag="st2")

    wt = [
        wpool.tile([128, TAPS, COUT], F32R, tag=f"wt{c}", name=f"wt{c}")
        for c in range(NCH)
    ]
    xs = []
    for c in range(NCH):
        xc = xpool.tile([128, H + 2, W + 2], F32R, tag=f"xs{c}", name=f"xs{c}")
        xs.append(xc)
    xst0 = xpool.tile([128, H, W], F32, tag="xst0", name="xst0")
    xst123 = xpool.tile([128, NCH - 1, PIX], F32, tag="xst123", name="xst123")

    warm_src = const.tile([1, 128], F32R, tag="warm_src")
    ones_r = const.tile([1, 1], F32R, tag="ones_r")
    warm_l = const.tile([128, 128], F32R, tag="warm_l")
    warm_r = const.tile([128, 512], F32R, tag="warm_r")

    # one PSUM bank shared by: warmup dst, sigma accumulation row, transposes
    sigbank = sigpsum.tile([128, 512], F32, tag="sigbank")
    sig_ps = sigbank[0:1, :]
    warm_ps = sigbank[0:1, 0:128]

    eps_b = const.tile([1, 1], F32, tag="eps_b")
    ones_t = const.tile([1, 1], F32, tag="ones_t")
    sqrt_dummy = const.tile([1, 1], F32, tag="sqrt_dummy")
    sig_sq = const.tile([1, COUT], F32, tag="sig_sq")
    sig_sd = const.tile([128, NCH], F32, tag="sig_sd")
    sig_t = const.tile([128, NCH], F32, tag="sig_t")
    # per-h output staging: oc writes cols [oc*512:(oc+1)*512]
    ob_h = [
        opool.tile([128, NCH * 512], F32, tag=f"ob{h}", name=f"ob{h}")
        for h in range(2)
    ]

    # --- early memsets (no data deps) ---
    nc.gpsimd.memset(warm_src[:].bitcast(F32), 1.0)
    nc.gpsimd.memset(ones_r[:].bitcast(F32), 1.0)
    nc.gpsimd.memset(warm_l[:].bitcast(F32), 0.0)
    nc.gpsimd.memset(warm_r[:].bitcast(F32), 0.0)
    nc.vector.memset(eps_b[:], EPS)
    nc.vector.memset(ones_t[:], 1.0)
    nc.vector.memset(sqrt_dummy[:], 1.0)
    for c in range(NCH):
        eng = nc.vector if c == 0 else nc.gpsimd
        v = xs[c][:].bitcast(F32)
        eng.memset(v[:, 0, :], 0.0)
        eng.memset(v[:, H + 1, :], 0.0)
        eng.memset(v[:, 1 : H + 1, 0], 0.0)
        eng.memset(v[:, 1 : H + 1, W + 1], 0.0)

    # --- DMA issue order. The 16 DMA engines are one shared pool: keep
    # per-partition segments >=4KB anywhere near the critical window. ---
    # gpsimd SWDGE ring starts executing ~0.7us before the HWDGE engines
    # clear their preamble barrier - the first weight taps ride it.
    nc.gpsimd.dma_start(wt[0][:, 0:3], wt_d[:, 0, 0:3])
    # sync ring
    nc.sync.dma_start(st[:], st_d[:])
    nc.sync.dma_start(wt[0][:, 3:9], wt_d[:, 0, 3:9])
    # scalar ring (parallel; its ACT table load does not block DMA issue).
    # x123 sits between x0 and wt1 so wt1 does not compete with wt0b's
    # window on the shared DMA-engine pool.
    nc.scalar.dma_start(xst0[:], x_d[0].rearrange("p (h w) -> p h w", h=H))
    nc.scalar.dma_start(xst123[:], x_d[1:NCH].rearrange("c p q -> p c q"))
    for c in range(1, NCH):
        nc.scalar.dma_start(wt[c][:], wt_d[:, c])

    # --- style scales (DVE; after style DMA) ---
    nc.vector.tensor_scalar_mul(st_rc[:], st[:], RC)
    nc.vector.tensor_mul(st2[:], st[:], st[:])

    # --- x modulation (DVE); x0 scaled in halves so conv's first taps can
    # start ~0.4us after x0 lands ---
    nc.vector.tensor_scalar_mul(
        xs[0][:, 1:17, 1 : W + 1], xst0[:, 0:16, :], st_rc[:, 0:1]
    )
    nc.vector.tensor_scalar_mul(
        xs[0][:, 17 : H + 1, 1 : W + 1], xst0[:, 16:32, :], st_rc[:, 0:1]
    )
    for c in range(1, NCH):
        nc.vector.tensor_scalar_mul(
            xs[c][:, 1 : H + 1, 1 : W + 1],
            xst123[:, c - 1].rearrange("p (h w) -> p h w", h=H),
            st_rc[:, c : c + 1],
        )

    # --- PE warmup: tiny matmuls bridge the p-state ramp until conv deps
    # land. Tiny ONLY: full-size primers pull the HAM grant earlier but
    # trigger duty-cycle oscillation that costs far more mid-stream. ---
    for _ in range(36):
        nc.tensor.matmul(warm_ps, ones_r[:], warm_src[:], start=True, stop=True)

    # wave A groups: (oc, h=0) -> banks pc0-3; wave B: g4-6 fresh banks, g7
    # reuses pc0 (freed by wave A's flush long before g7 starts)
    wave_a = [(oc, 0) for oc in range(NCH)]
    wave_b = [(oc, 1) for oc in range(NCH)]
    pc = {
        g: psum.tile([128, 512], F32, tag=f"pc{i}", name=f"pc{i}")
        for i, g in enumerate(wave_a)
    }

    def conv_mm(g, c, t, start, stop):
        oc, h = g
        dy, dx = t // 3, t % 3
        h0 = h * 16
        nc.tensor.matmul(
            pc[g][:],
            wt[c][:, t, oc * 128 : (oc + 1) * 128],
            xs[c][:, dy + h0 : dy + h0 + 16, dx : dx + W],
            start=start,
            stop=stop,
        )

    # Per-chunk sum over taps of squared weights (ACT squares, DVE adds):
    # cuts the PE cost of the sigma reduction from 36 matmuls to 4.
    w2s = {}

    def sig_squares(c):
        parts = []
        for t in range(TAPS):
            w2 = sqpool.tile([128, COUT], BF16, tag=f"w2_{t % 3}", name="w2")
            nc.scalar.activation(w2[:], wt[c][:, t], AF.Square)
            parts.append(w2)
            if t == 1:
                acc = sqpool.tile([128, COUT], BF16, tag=f"w2s{c}", name="w2s")
                nc.vector.tensor_add(acc[:], parts[0][:], parts[1][:])
            elif t > 1:
                nc.vector.tensor_add(acc[:], acc[:], parts[-1][:])
        w2s[c] = acc
        if c == NCH - 1:
            # prefetch the Sqrt ACT table (1.3us load) off the critical path
            nc.scalar.activation(sqrt_dummy[:], sqrt_dummy[:], AF.Sqrt)

    def sig_mm(c):
        nc.tensor.matmul(
            sig_ps, st2[:, c : c + 1], w2s[c][:], start=(c == 0), stop=(c == NCH - 1)
        )

    def sig_finalize():
        # sqrt(RC^2*q + eps) [1,512] -> PE-transpose -> [128,4] -> reciprocal
        nc.scalar.activation(
            sig_sq[:], sig_ps, AF.Sqrt, bias=eps_b[:], scale=RC * RC
        )
        for oc in range(NCH):
            nc.tensor.transpose(
                sigbank[:, oc : oc + 1],
                sig_sq[0:1, oc * 128 : (oc + 1) * 128],
                ones_t[:],
            )
        nc.vector.tensor_copy(sig_sd[:], sigbank[:, 0:NCH])
        nc.vector.reciprocal(sig_t[:], sig_sd[:])

    def flush(g):
        # PSUM -> SBUF scaled by sigma_inv, on DVE; waits on sig_t which lands
        # mid-wave-B - only g7's bank reuse (~26us later) depends on it.
        oc, h = g
        nc.vector.tensor_scalar_mul(
            ob_h[h][:, oc * 512 : (oc + 1) * 512], pc[g][:], sig_t[:, oc : oc + 1]
        )

    def out_dma(h, pair):
        nc.sync.dma_start(
            out_d[h, pair, :, :],
            ob_h[h][:, pair * 1024 : (pair + 1) * 1024].rearrange(
                "p (a q) -> p a q", a=2
            ),
        )

    # --- wave A (h=0): tap-major, group-minor; squares pipeline alongside ---
    for c in range(NCH):
        sig_squares(c)
        for t in range(TAPS):
            for g in wave_a:
                conv_mm(g, c, t, c == 0 and t == 0, c == NCH - 1 and t == TAPS - 1)

    # --- wave B (h=1): per-group serial chains; sigma reduction at g4 chunk
    # boundaries, finalize after g4. All flush emissions come after
    # sig_finalize so sig_t's write precedes every flush in DVE program
    # order (the tile tracker cannot order a read before a future write). ---
    for gi, g in enumerate(wave_b):
        tag = "pc0" if gi == 3 else f"pcB{gi}"
        pc[g] = psum.tile([128, 512], F32, tag=tag, name=f"pcb{gi}")
        k = 0
        for c in range(NCH):
            if gi == 0:
                sig_mm(c)
            for t in range(TAPS):
                conv_mm(g, c, t, k == 0, k == TAPS * NCH - 1)
                k += 1
        if gi == 0:
            sig_finalize()
            for ga in wave_a:
                flush(ga)
            out_dma(0, 0)
            out_dma(0, 1)
        flush(g)
        if gi == 1:
            out_dma(1, 0)
        elif gi == 3:
            out_dma(1, 1)


_CACHE = None


def _get_compiled():
    global _CACHE
    if _CACHE is None:
        nc = bacc.Bacc(
            "TRN2", target_bir_lowering=False, debug=False, num_devices=B
        )
        x_d = nc.dram_tensor("x", [NCH, 128, PIX], F32, kind="ExternalInput").ap()
        st_d = nc.dram_tensor("style", [128, NCH], F32, kind="ExternalInput").ap()
        wt_d = nc.dram_tensor(
            "wt", [128, NCH, TAPS, COUT], F32R, kind="ExternalInput"
        ).ap()
        # out layout: [h, oc_pair, 128, 2, 512] -> host reassembles
        out_d = nc.dram_tensor(
            "out", [2, 2, 128, 2, 512], F32, kind="ExternalOutput"
        ).ap()
        with tile.TileContext(nc) as tc, ExitStack() as ctx:
            _body(ctx, tc, x_d, st_d, wt_d, out_d)
        nc.compile()
        _CACHE = nc
    return _CACHE


def kernel(x, style, weight):
    """x: (8,512,32,32) f32, style: (8,512) f32, weight: (512,512,3,3) f32
    -> (8,512,32,32) f32"""
    global LAST_RESULTS
    x = np.ascontiguousarray(np.asarray(x, dtype=np.float32))
    style = np.asarray(style, dtype=np.float32)
    weight = np.asarray(weight, dtype=np.float32)

    # Host-side layout only (no arithmetic): lhsT weight layout
    # wt[i_lo, c, t, o] = weight[o, c*128 + i_lo, t//3, t%3]
    wt = np.ascontiguousarray(
        weight.reshape(COUT, NCH, 128, TAPS).transpose(2, 1, 3, 0)
    )
    in_maps = []
    for b in range(B):
        in_maps.append(
            {
                "x": x[b].reshape(NCH, 128, PIX),
                "style": np.ascontiguousarray(style[b].reshape(NCH, 128).T),
                "wt": wt,
            }
        )

    nc = _get_compiled()
    res = run_bass_kernel_spmd(nc, in_maps, list(range(B)), trace=TRACE)
    LAST_RESULTS = res
    out = np.empty((B, COUT, H, W), dtype=np.float32)
    for b in range(B):
        # out HBM [h, pair, 128(cout_lo), j(oc in pair), 512(pix half)]
        o = res.results[b]["out"]
        o = o.transpose(1, 3, 2, 0, 4)  # -> [pair, j, cout_lo, h, q]
        out[b] = o.reshape(COUT, H, W)
    return out
